# revision 46
# baseline (speedup 1.0000x reference)
"""Trainium2 Bass kernel for nn_EventPairCompositionModel.

Strategy (data-parallel over batch, 8 cores, B=512 -> 64 per core):
  - Host compacts the 60MB f32 table per core to the ~24K unique rows its
    shard touches, stored fp8e4m3 (x16 scale), rows padded to 512 elems
    (512B), indices remapped to int16.  SWDGE dma_gather (transpose mode)
    lands rows K-major as 16-bit token pairs: partition p holds elems
    (256f+2p, 256f+2p+1) at free bytes (2i, 2i+1).
  - MLP1/MLP2 run in fp8 with MatmulPerfMode.DoubleRow (256-K per pass,
    2x PE throughput).  The required K/M permutations are absorbed into
    host-side weight layouts; scales (x16 per operand) are folded into
    the activation scale (1/256) and biases.
  - s1 activations write fp8 interleaved pairs (stride-2 bytes) so MLP2
    consumes them directly in DoubleRow layout.
  - Cosine numerators/denominators via small per-b bf16 matmuls that land
    n-on-partitions; norms folded together through one exp(-0.5 ln x).
  - KNRM kernel pooling via ones-matmul partition reductions, distance
    kernel path, final linear + sigmoid, all on-chip.
  - If a shard ever touches >32767 unique rows (can't happen for random
    inputs), falls back to a slow indirect-DMA gather of the full table.
All 8 cores run the identical program on their own batch shard (SPMD, no
collectives); host concatenates the 8 (64,1) outputs.
"""

import numpy as np
import ml_dtypes

import concourse.bacc as bacc
import concourse.bass as bass
import concourse.tile as tile
import concourse.mybir as mybir
from concourse.bass import IndirectOffsetOnAxis
from concourse.bass_utils import run_bass_kernel_spmd
from concourse import library_config

F32 = mybir.dt.float32
BF16 = mybir.dt.bfloat16
F8 = mybir.dt.float8e4
I16 = mybir.dt.int16
I32 = mybir.dt.int32
AF = mybir.ActivationFunctionType
DR = mybir.MatmulPerfMode.DoubleRow

# Problem shapes (hardcoded per spec)
B, N, C, E = 512, 128, 4, 300
V = 50000
H1, H2 = 512, 256
NF, NK = 8, 11
NCORES = 8
BC = B // NCORES          # 64 batches per core
EP = 512                  # padded fp8 row length (512B, two 256-elem chunks)
CE = C * EP               # padded K (2048); real K is 4*300=1200
CT = 32768                # compact table rows (int16-indexable)
GROUPS = (BC * N) // 512  # 16 groups of 512 (b,n) pairs
SUBT = 4                  # 128-pair subtiles per group (s <-> b offset)
EB = 128                  # event-path width (64 real b + 64 junk)
FS8 = 16.0                # fp8 operand scale

MUS = [1.0, 0.9, 0.7, 0.5, 0.3, 0.1, -0.1, -0.3, -0.5, -0.7, -0.9]
SIGMAS = [1e-3] + [0.1] * 10

_PROGRAM_CACHE = {}


def _build_program(fast: bool):
    if fast in _PROGRAM_CACHE:
        return _PROGRAM_CACHE[fast]

    nc = bacc.Bacc("TRN2", target_bir_lowering=False, debug=False, num_swdge_queues=4)

    # ---- DRAM I/O ----
    if fast:
        ctab = nc.dram_tensor("ctab", (CT, EP), F8, kind="ExternalInput")
        cidx = nc.dram_tensor("cidx", (128, GROUPS * C * 32), I16, kind="ExternalInput")
        xev = nc.dram_tensor("xev", (128, 2048), F8, kind="ExternalInput")
        cblob = nc.dram_tensor("cblob", (128, 148), F32, kind="ExternalInput")
        w1t = nc.dram_tensor("w1t", (128, 16 * H1), F8, kind="ExternalInput")
        w2t = nc.dram_tensor("w2t", (128, 4 * H2), F8, kind="ExternalInput")
        wvt = nc.dram_tensor("wvt", (128, 4 * 9), F8, kind="ExternalInput")
    else:
        ctab = nc.dram_tensor("table", (V + 1, E), F32, kind="ExternalInput")
        cidx = nc.dram_tensor("ctxidx", (128, BC * C), I32, kind="ExternalInput")
        eidx = nc.dram_tensor("evidx", (BC, C), I32, kind="ExternalInput")
        w1t = nc.dram_tensor("w1t", (4 * 384, H1), BF16, kind="ExternalInput")
        w2t = nc.dram_tensor("w2t", (H1, H2), BF16, kind="ExternalInput")
        wvt = nc.dram_tensor("wvt", (4 * 384, 9), BF16, kind="ExternalInput")
        b1d = nc.dram_tensor("b1d", (128, 4), F32, kind="ExternalInput")
        b2d = nc.dram_tensor("b2d", (128, 2), F32, kind="ExternalInput")
        bvd = nc.dram_tensor("bvd", (9, 1), F32, kind="ExternalInput")
        wct = nc.dram_tensor("wct", (128, 1), F32, kind="ExternalInput")
        wckp = nc.dram_tensor("wckp", (1, NK), F32, kind="ExternalInput")
        bcd = nc.dram_tensor("bcd", (1, 1), F32, kind="ExternalInput")
        ndsq = nc.dram_tensor("ndsq", (9, BC), F32, kind="ExternalInput")
        featT = nc.dram_tensor("featT", (NF, BC), F32, kind="ExternalInput")
    out_d = nc.dram_tensor("out", (BC, 1), F32, kind="ExternalOutput")

    with tile.TileContext(nc) as tc:
        with (
            tc.tile_pool(name="consts", bufs=1) as cpool,
            tc.tile_pool(name="xg", bufs=4) as xgpool,
            tc.tile_pool(name="xt", bufs=3) as xtpool,
            tc.tile_pool(name="s1", bufs=4) as s1pool,
            tc.tile_pool(name="s2", bufs=4) as s2pool,
            tc.tile_pool(name="csq", bufs=4) as csqpool,
            tc.tile_pool(name="small", bufs=2) as smpool,
            tc.tile_pool(name="pm1", bufs=3, space="PSUM") as pm1,
            tc.tile_pool(name="pm2", bufs=2, space="PSUM") as pm2,
            tc.tile_pool(name="ptn", bufs=1, space="PSUM") as ptn,
            tc.tile_pool(name="pmisc", bufs=2, space="PSUM") as pmisc,
        ):
            # ---- load constants ----
            nc.gpsimd.load_library(library_config.mlp)
            if fast:
                cidx_s = cpool.tile([128, GROUPS * C * 32], I16)
                nc.sync.dma_start(cidx_s[:], cidx.ap())
                w1t_s = cpool.tile([128, 16 * H1], F8)
                nc.scalar.dma_start(w1t_s[:], w1t.ap())
                xeT = cpool.tile([128, 2048], F8)
                nc.sync.dma_start(xeT[:], xev.ap())
                blob_s = cpool.tile([128, 148], F32)
                nc.sync.dma_start(blob_s[:], cblob.ap())
                w2t_s = cpool.tile([128, 4 * H2], F8)
                nc.scalar.dma_start(w2t_s[:], w2t.ap())
                wvt_s = cpool.tile([128, 4 * 9], F8)
                nc.scalar.dma_start(wvt_s[:], wvt.ap())
            else:
                cidx_s = cpool.tile([128, BC * C], I32)
                nc.sync.dma_start(cidx_s[:], cidx.ap())
                eidx_s = cpool.tile([BC, C], I32)
                nc.sync.dma_start(eidx_s[:], eidx.ap())
                w1t_s = cpool.tile([128, 12 * H1], BF16)
                nc.sync.dma_start(
                    w1t_s[:].rearrange("p (t m) -> p t m", t=12),
                    w1t.ap().rearrange("(t p) m -> p t m", p=128),
                )
                w2t_s = cpool.tile([128, 4 * H2], BF16)
                nc.scalar.dma_start(
                    w2t_s[:].rearrange("p (t m) -> p t m", t=4),
                    w2t.ap().rearrange("(t p) m -> p t m", p=128),
                )
                wvt_s = cpool.tile([128, 12 * 9], BF16)
                nc.scalar.dma_start(
                    wvt_s[:].rearrange("p (t m) -> p t m", t=12),
                    wvt.ap().rearrange("(t p) m -> p t m", p=128),
                )
            b1_s = cpool.tile([128, 4], F32)
            b2_s = cpool.tile([128, 2], F32)
            bv_s = cpool.tile([9, 1], F32)
            wct_s = cpool.tile([128, 1], F32)
            wckp_s = cpool.tile([1, NK], F32)
            bc_s = cpool.tile([1, 1], F32)
            ndsq_s = cpool.tile([9, BC], F32)
            feat_s = cpool.tile([128, BC], F32)
            nc.vector.memset(feat_s[:], 0.0)
            if fast:
                nc.vector.tensor_scalar_add(b1_s[:], blob_s[:, 0:4], 0.0)
                nc.vector.tensor_scalar_add(b2_s[:], blob_s[:, 4:6], 0.0)
                nc.vector.tensor_scalar_add(bv_s[:], blob_s[0:9, 6:7], 0.0)
                nc.vector.tensor_scalar_add(wct_s[:], blob_s[:, 7:8], 0.0)
                nc.vector.tensor_scalar_add(wckp_s[:], blob_s[0:1, 8:19], 0.0)
                nc.vector.tensor_scalar_add(bc_s[:], blob_s[0:1, 19:20], 0.0)
                nc.vector.tensor_scalar_add(ndsq_s[:], blob_s[0:9, 20:84], 0.0)
                nc.vector.tensor_scalar_add(
                    feat_s[64 : 64 + NF, :], blob_s[0:NF, 84:148], 0.0
                )
            else:
                nc.sync.dma_start(b1_s[:], b1d.ap())
                nc.sync.dma_start(b2_s[:], b2d.ap())
                nc.sync.dma_start(bv_s[:], bvd.ap())
                nc.sync.dma_start(wct_s[:], wct.ap())
                nc.sync.dma_start(wckp_s[:], wckp.ap())
                nc.sync.dma_start(bc_s[:], bcd.ap())
                nc.sync.dma_start(ndsq_s[:], ndsq.ap())
                nc.sync.dma_start(feat_s[64 : 64 + NF, :], featT.ap())
            ones_s = cpool.tile([128, 1], BF16)
            nc.vector.memset(ones_s[:], 1.0)
            if fast:
                # DVFS warm-up: keep the PE busy during the gpsimd library
                # load so the clock is ramped when the first (serial,
                # latency-critical) gathers and matmuls run.
                warmm = cpool.tile([128, 512], BF16)
                nc.vector.memset(warmm[:], 1.0)
                for _ in range(24):
                    pwarm = pmisc.tile([1, 512], F32, tag="pmisc", name="pwarm")
                    nc.tensor.matmul(
                        pwarm[:], ones_s[:], warmm[:], start=True, stop=True
                    )
            onesrow_s = cpool.tile([1, 128], F32)
            nc.vector.memset(onesrow_s[:], 1.0)
            onesf_s = cpool.tile([128, 1], F32)
            nc.vector.memset(onesf_s[:], 1.0)
            eps_s = cpool.tile([128, 1], F32)
            nc.vector.memset(eps_s[:], 1e-20)
            mub_s = cpool.tile([128, NK], F32)
            coef_s = cpool.tile([128, NK], F32)
            for k in range(NK):
                nc.vector.memset(mub_s[:, k : k + 1], -MUS[k])
                nc.vector.memset(
                    coef_s[:, k : k + 1], -1.0 / (2.0 * SIGMAS[k] ** 2)
                )

            eh2 = [
                cpool.tile([128, EB], BF16, tag=f"eh2_{k}", name=f"eh2_{k}")
                for k in range(2)
            ]

            if fast:
                # ---- fp8 DoubleRow access-pattern helpers ----
                def w1_lhs(c, f, m):
                    # [p, beta(2), mm(128)] for K-chunk (c,f), M-tile m
                    v = w1t_s[:].rearrange(
                        "p (cf beta u) -> p cf beta u", beta=2, u=H1
                    )
                    return v[:, c * 2 + f, :, 128 * m : 128 * (m + 1)]

                def w2_lhs(P, m):
                    v = w2t_s[:].rearrange(
                        "p (q beta u) -> p q beta u", beta=2, u=H2
                    )
                    return v[:, P, :, 128 * m : 128 * (m + 1)]

                def wv_lhs(f):
                    v = wvt_s[:].rearrange(
                        "p (f beta u) -> p f beta u", f=2, u=9
                    )
                    return v[:, f, :, :]

                def issue_group_gather(g):
                    xt = xtpool.tile([128, C * 2048], F8, tag="xt", name="xt")
                    for c in range(C):
                        nc.gpsimd.dma_gather(
                            out_ap=xt[:].rearrange(
                                "p (c j i) -> p c j i", c=C, j=4
                            )[:, c, :, :],
                            in_ap=ctab.ap(),
                            idxs_ap=cidx_s[
                                :, 32 * (C * g + c) : 32 * (C * g + c + 1)
                            ],
                            num_idxs=512,
                            num_idxs_reg=512,
                            elem_size=EP,
                            transpose=True,
                        )
                    return xt


                def xe_rhs(c, f):
                    # [p, beta(2), n(128)]: comp c chunk f over event cols
                    v = xeT[:].rearrange(
                        "p (f i beta) -> p f beta i", f=2, beta=2
                    )
                    return v[:, f, :, 128 * c : 128 * (c + 1)]

                s1e = cpool.tile([128, 512], F8)
                for m in range(4):
                    pe = pmisc.tile([128, EB], F32, tag="pmisc", name="pe")
                    kk = 0
                    for c in range(C):
                        for f in range(2):
                            nc.tensor.matmul(
                                pe[:], w1_lhs(c, f, m), xe_rhs(c, f),
                                start=(kk == 0), stop=(kk == 7), perf_mode=DR,
                            )
                            kk += 1
                    P, bb = divmod(m, 2)
                    o = s1e[:].rearrange(
                        "p (q i beta) -> p q beta i", q=2, beta=2
                    )[:, P, bb, :]
                    nc.scalar.activation(
                        o, pe[:], AF.Relu, bias=b1_s[:, m : m + 1],
                        scale=1.0 / FS8,
                    )

                def s1e_rhs(P):
                    v = s1e[:].rearrange(
                        "p (q i beta) -> p q beta i", q=2, beta=2
                    )
                    return v[:, P, :, :]

                for m in range(2):
                    pe2 = pmisc.tile([128, EB], F32, tag="pmisc", name="pe2")
                    for P in range(2):
                        nc.tensor.matmul(
                            pe2[:], w2_lhs(P, m), s1e_rhs(P),
                            start=(P == 0), stop=(P == 1), perf_mode=DR,
                        )
                    nc.scalar.activation(
                        eh2[m][:], pe2[:], AF.Relu, bias=b2_s[:, m : m + 1],
                        scale=1.0 / (FS8 * FS8),
                    )

                # variances -> dist_emb rows 32..40 of feat_s
                # (plain fp8 matmuls; DoubleRow rejects the 9-wide ldweights)
                pv = pmisc.tile([9, EB], F32, tag="pmisc", name="pv")
                for f in range(2):
                    for bb in range(2):
                        nc.tensor.matmul(
                            pv[:],
                            wv_lhs(f)[:, bb, :],
                            xe_rhs(1, f)[:, bb, :],
                            start=(f == 0 and bb == 0),
                            stop=(f == 1 and bb == 1),
                        )
                ez_s = smpool.tile([9, EB], F32)
                nc.scalar.activation(
                    ez_s[:], pv[:], AF.Exp, bias=bv_s[:],
                    scale=1.0 / (FS8 * FS8),
                )
            else:
                # ---- slow-path event MLP (bf16, padded-384 K layout) ----
                xe = cpool.tile([EB, 4 * 384], BF16)
                nc.vector.memset(xe[:], 0.0)
                nc.gpsimd.indirect_dma_start(
                    out=xe[0:BC, :].rearrange("p (c e) -> p c e", c=C)[:, :, 0:E],
                    out_offset=None,
                    in_=ctab.ap(),
                    in_offset=IndirectOffsetOnAxis(ap=eidx_s[:], axis=0),
                )
                xeT = cpool.tile([128, 12 * EB], BF16)
                nc.sync.dma_start_transpose(
                    xeT[:].rearrange("p (j i) -> p j i", j=12), xe[:]
                )

                def xeT_k(j):
                    return xeT[:, EB * j : EB * (j + 1)]

                s1e = cpool.tile([128, 4 * EB], BF16)
                for m in range(4):
                    pe = pmisc.tile([128, EB], F32, tag="pmisc", name="pe")
                    for j in range(12):
                        nc.tensor.matmul(
                            pe[:],
                            w1t_s[:, H1 * j + 128 * m : H1 * j + 128 * m + 128],
                            xeT_k(j),
                            start=(j == 0),
                            stop=(j == 11),
                        )
                    nc.scalar.activation(
                        s1e[:, EB * m : EB * (m + 1)], pe[:], AF.Relu,
                        bias=b1_s[:, m : m + 1],
                    )

                for m in range(2):
                    pe2 = pmisc.tile([128, EB], F32, tag="pmisc", name="pe2")
                    for j in range(4):
                        nc.tensor.matmul(
                            pe2[:],
                            w2t_s[:, H2 * j + 128 * m : H2 * j + 128 * m + 128],
                            s1e[:, EB * j : EB * (j + 1)],
                            start=(j == 0),
                            stop=(j == 3),
                        )
                    nc.scalar.activation(
                        eh2[m][:], pe2[:], AF.Relu, bias=b2_s[:, m : m + 1]
                    )

                pv = pmisc.tile([9, EB], F32, tag="pmisc", name="pv")
                for j in range(12):
                    nc.tensor.matmul(
                        pv[:],
                        wvt_s[:, 9 * j : 9 * (j + 1)],
                        xeT_k(j),
                        start=(j == 0),
                        stop=(j == 11),
                    )
                ez_s = smpool.tile([9, EB], F32)
                nc.scalar.activation(ez_s[:], pv[:], AF.Exp, bias=bv_s[:])

            # softplus -> variances -> dist kernel features (shared)
            ez1_s = smpool.tile([9, EB], F32)
            nc.vector.tensor_scalar_add(ez1_s[:], ez_s[:], 1.0)
            var_s = smpool.tile([9, EB], F32)
            nc.scalar.activation(var_s[:], ez1_s[:], AF.Ln)
            rv_s = smpool.tile([9, EB], F32)
            nc.vector.reciprocal(rv_s[:], var_s[:])
            q_s = smpool.tile([9, BC], F32)
            nc.vector.tensor_mul(q_s[:], ndsq_s[:], rv_s[:, 0:BC])
            nc.scalar.activation(feat_s[32:41, :], q_s[:], AF.Exp)

            # |e|^2 per b, broadcast to all 128 partitions via outer product
            esq = [
                smpool.tile([128, EB], BF16, tag=f"esq_{k}", name=f"esq_{k}")
                for k in range(2)
            ]
            for k in range(2):
                nc.vector.tensor_mul(esq[k][:], eh2[k][:], eh2[k][:])
            pne = pmisc.tile([1, EB], F32, tag="pmisc", name="pne")
            for k in range(2):
                nc.tensor.matmul(
                    pne[:], ones_s[:], esq[k][:], start=(k == 0), stop=(k == 1)
                )
            ne2_s = smpool.tile([1, BC], F32)
            nc.scalar.copy(ne2_s[:], pne[:, 0:BC])
            pne2bc = pmisc.tile([128, BC], F32, tag="pmisc", name="pne2bc")
            nc.tensor.matmul(
                pne2bc[:], onesrow_s[:], ne2_s[:], start=True, stop=True
            )
            ne2bc_s = cpool.tile([128, BC], F32)
            nc.scalar.copy(ne2bc_s[:], pne2bc[:])

            # feature-vector part of the score (ready before the groups)
            psc = pmisc.tile([1, BC], F32, tag="pmisc", name="psc")
            nc.tensor.matmul(psc[:], wct_s[:], feat_s[:], start=True, stop=True)
            psc_s = cpool.tile([1, BC], F32)
            nc.vector.tensor_scalar_add(psc_s[:], psc[:], 0.0)

            # persistent SBUF accumulators, n on partitions, b on free
            traw_s = cpool.tile([128, BC], F32)
            ncsq_s = cpool.tile([128, BC], F32)
            kps_s = cpool.tile([1, BC], F32)

            def do_pool(b0, b1):
                """RBF kernel pooling for b in [b0, b1)."""
                QB = b1 - b0
                sl = slice(b0, b1)
                prodn = smpool.tile([128, QB], F32, tag="prodn", name="prodn")
                nc.vector.tensor_mul(prodn[:], ncsq_s[:, sl], ne2bc_s[:, sl])
                lnp = smpool.tile([128, QB], F32, tag="lnp", name="lnp")
                nc.scalar.activation(lnp[:], prodn[:], AF.Ln, bias=eps_s[:])
                nrmf = smpool.tile([128, QB], F32, tag="nrmf", name="nrmf")
                nc.scalar.activation(nrmf[:], lnp[:], AF.Exp, scale=-0.5)
                transq = smpool.tile([128, QB], F32, tag="transq", name="transq")
                nc.vector.tensor_mul(transq[:], traw_s[:, sl], nrmf[:])
                dkb = smpool.tile([128, NK * QB], F32, tag="dkb", name="dkb")
                nc.vector.tensor_tensor(
                    out=dkb[:].rearrange("p (k b) -> p k b", k=NK),
                    in0=transq[:][:, None, :].broadcast_to([128, NK, QB]),
                    in1=mub_s[:][:, :, None].broadcast_to([128, NK, QB]),
                    op=mybir.AluOpType.add,
                )
                sqkb = smpool.tile([128, NK * QB], F32, tag="sqkb", name="sqkb")
                nc.vector.tensor_mul(sqkb[:], dkb[:], dkb[:])
                argkb = smpool.tile([128, NK * QB], F32, tag="argkb", name="argkb")
                nc.vector.tensor_tensor(
                    out=argkb[:].rearrange("p (k b) -> p k b", k=NK),
                    in0=sqkb[:].rearrange("p (k b) -> p k b", k=NK),
                    in1=coef_s[:][:, :, None].broadcast_to([128, NK, QB]),
                    op=mybir.AluOpType.mult,
                )
                argc = smpool.tile([128, NK * QB], F32, tag="argc", name="argc")
                nc.vector.tensor_scalar_max(argc[:], argkb[:], -87.0)
                ekb = smpool.tile([128, NK * QB], BF16, tag="ekb", name="ekb")
                nc.scalar.activation(ekb[:], argc[:], AF.Exp)
                kpc = smpool.tile([1, NK * QB], F32, tag="kpc", name="kpc")
                for lo in range(0, NK * QB, 512):
                    hi = min(lo + 512, NK * QB)
                    pp = pmisc.tile([1, hi - lo], F32, tag="pmisc", name="pp")
                    nc.tensor.matmul(
                        pp[:], ones_s[:], ekb[:, lo:hi], start=True, stop=True
                    )
                    nc.vector.tensor_scalar_max(kpc[:, lo:hi], pp[:], 1e-10)
                kpl = smpool.tile([1, NK * QB], F32, tag="kpl", name="kpl")
                nc.scalar.activation(kpl[:], kpc[:], AF.Ln)
                kpw = smpool.tile([1, QB * NK], F32, tag="kpw", name="kpw")
                nc.vector.tensor_tensor(
                    out=kpw[:].rearrange("p (b k) -> p b k", b=QB),
                    in0=kpl[:].rearrange("p (k b) -> p b k", k=NK),
                    in1=wckp_s[:][:, None, :].broadcast_to([1, QB, NK]),
                    op=mybir.AluOpType.mult,
                )
                nc.vector.reduce_sum(
                    out=kps_s[:, sl],
                    in_=kpw[:].rearrange("p (b k) -> p b k", b=QB),
                    axis=mybir.AxisListType.X,
                )

            # ---- context groups ----
            for g in range(GROUPS):
                if fast:
                    xt = issue_group_gather(g)

                    def xt_rhs(c, f):
                        v = xt[:].rearrange(
                            "p (c f i beta) -> p c f beta i", c=C, f=2, beta=2
                        )
                        return v[:, c, f, :, :]

                    s1 = s1pool.tile([128, 2048], F8, tag="s1", name="s1")
                    for m in range(4):
                        p1 = pm1.tile([128, 512], F32)
                        kk = 0
                        for c in range(C):
                            for f in range(2):
                                nc.tensor.matmul(
                                    p1[:], w1_lhs(c, f, m), xt_rhs(c, f),
                                    start=(kk == 0), stop=(kk == 7),
                                    perf_mode=DR,
                                )
                                kk += 1
                        P, bb = divmod(m, 2)
                        o = s1[:].rearrange(
                            "p (q i beta) -> p q beta i", q=2, beta=2
                        )[:, P, bb, :]
                        nc.scalar.activation(
                            o, p1[:], AF.Relu, bias=b1_s[:, m : m + 1],
                            scale=1.0 / FS8,
                        )

                    def s1_rhs(P):
                        v = s1[:].rearrange(
                            "p (q i beta) -> p q beta i", q=2, beta=2
                        )
                        return v[:, P, :, :]

                    s2 = [
                        s2pool.tile(
                            [128, 512], BF16, tag=f"s2_{m}", name=f"s2_{m}"
                        )
                        for m in range(2)
                    ]
                    for m in range(2):
                        p2 = pm2.tile([128, 512], F32)
                        for P in range(2):
                            nc.tensor.matmul(
                                p2[:], w2_lhs(P, m), s1_rhs(P),
                                start=(P == 0), stop=(P == 1), perf_mode=DR,
                            )
                        nc.scalar.activation(
                            s2[m][:], p2[:], AF.Relu, bias=b2_s[:, m : m + 1],
                            scale=1.0 / (FS8 * FS8),
                        )

                    csq = [
                        csqpool.tile(
                            [128, 512], BF16, tag=f"csq_{m}", name=f"csq_{m}"
                        )
                        for m in range(2)
                    ]
                    for m in range(2):
                        nc.vector.tensor_mul(csq[m][:], s2[m][:], s2[m][:])

                    # raw dots and |c|^2, n on partitions, one column per b
                    pTN = ptn.tile([128, 2 * SUBT], F32, tag="pTN", name="pTN")
                    pT = pTN[:, 0:SUBT]
                    pN = pTN[:, SUBT : 2 * SUBT]
                    for s in range(SUBT):
                        b = SUBT * g + s
                        for k in range(2):
                            nc.tensor.matmul(
                                pT[:, s : s + 1],
                                s2[k][:, 128 * s : 128 * (s + 1)],
                                eh2[k][:, b : b + 1],
                                start=(k == 0),
                                stop=(k == 1),
                            )
                        for k in range(2):
                            nc.tensor.matmul(
                                pN[:, s : s + 1],
                                csq[k][:, 128 * s : 128 * (s + 1)],
                                ones_s[:],
                                start=(k == 0),
                                stop=(k == 1),
                            )
                else:
                    xg = xgpool.tile([128, SUBT * 4 * 384], BF16)
                    nc.vector.memset(
                        xg[:].rearrange("p (q e) -> p q e", e=384)[:, :, E:384],
                        0.0,
                    )
                    for s in range(SUBT):
                        nc.gpsimd.indirect_dma_start(
                            out=xg[:]
                            .rearrange("p (q c e) -> p q c e", q=SUBT, c=C)[
                                :, s, :, 0:E
                            ],
                            out_offset=None,
                            in_=ctab.ap(),
                            in_offset=IndirectOffsetOnAxis(
                                ap=cidx_s[
                                    :, (SUBT * g + s) * C : (SUBT * g + s + 1) * C
                                ],
                                axis=0,
                            ),
                        )
                    xt = xtpool.tile([128, 12 * 512], BF16)
                    for s in range(SUBT):
                        nc.sync.dma_start_transpose(
                            xt[:].rearrange(
                                "p (j z i) -> p j z i", j=12, z=SUBT
                            )[:, :, s, :],
                            xg[:, 1536 * s : 1536 * (s + 1)],
                        )

                    def xt_k(j):
                        return xt[:, 512 * j : 512 * (j + 1)]

                    s1t = [
                        s1pool.tile(
                            [128, 512], BF16, tag=f"s1_{m}", name=f"s1_{m}"
                        )
                        for m in range(4)
                    ]
                    for m in range(4):
                        p1 = pm1.tile([128, 512], F32)
                        for j in range(12):
                            nc.tensor.matmul(
                                p1[:],
                                w1t_s[
                                    :, H1 * j + 128 * m : H1 * j + 128 * m + 128
                                ],
                                xt_k(j),
                                start=(j == 0),
                                stop=(j == 11),
                            )
                        nc.scalar.activation(
                            s1t[m][:], p1[:], AF.Relu, bias=b1_s[:, m : m + 1]
                        )

                    s2 = [
                        s2pool.tile(
                            [128, 512], BF16, tag=f"s2_{m}", name=f"s2_{m}"
                        )
                        for m in range(2)
                    ]
                    for m in range(2):
                        p2 = pm2.tile([128, 512], F32)
                        for j in range(4):
                            nc.tensor.matmul(
                                p2[:],
                                w2t_s[
                                    :, H2 * j + 128 * m : H2 * j + 128 * m + 128
                                ],
                                s1t[j][:],
                                start=(j == 0),
                                stop=(j == 3),
                            )
                        nc.scalar.activation(
                            s2[m][:], p2[:], AF.Relu, bias=b2_s[:, m : m + 1]
                        )

                    csq = [
                        csqpool.tile(
                            [128, 512], BF16, tag=f"csq_{m}", name=f"csq_{m}"
                        )
                        for m in range(2)
                    ]
                    for m in range(2):
                        nc.vector.tensor_mul(csq[m][:], s2[m][:], s2[m][:])

                    # raw dots and |c|^2, n on partitions, one column per b
                    pTN = ptn.tile([128, 2 * SUBT], F32, tag="pTN", name="pTN")
                    pT = pTN[:, 0:SUBT]
                    pN = pTN[:, SUBT : 2 * SUBT]
                    for s in range(SUBT):
                        b = SUBT * g + s
                        for k in range(2):
                            nc.tensor.matmul(
                                pT[:, s : s + 1],
                                s2[k][:, 128 * s : 128 * (s + 1)],
                                eh2[k][:, b : b + 1],
                                start=(k == 0),
                                stop=(k == 1),
                            )
                        for k in range(2):
                            nc.tensor.matmul(
                                pN[:, s : s + 1],
                                csq[k][:, 128 * s : 128 * (s + 1)],
                                ones_s[:],
                                start=(k == 0),
                                stop=(k == 1),
                            )
                nc.vector.tensor_scalar_add(
                    traw_s[:, SUBT * g : SUBT * (g + 1)], pT, 0.0
                )
                nc.vector.tensor_scalar_add(
                    ncsq_s[:, SUBT * g : SUBT * (g + 1)], pN, 0.0
                )
                if fast and g == GROUPS - 2:
                    # prime the Ln/Exp activation tables during the last
                    # group's matmul window so the pooling tail finds them
                    # resident (saves 2x 1.3us table loads on the critical path)
                    dex = smpool.tile([1, 1], F32, tag="dex", name="dex")
                    nc.scalar.activation(dex[:], eps_s[0:1, :], AF.Exp)
                    dln = smpool.tile([1, 1], F32, tag="dln", name="dln")
                    nc.scalar.activation(dln[:], eps_s[0:1, :], AF.Ln)
            do_pool(0, BC // 2)
            do_pool(BC // 2, BC)

            # ---- final score ----
            tot_s = smpool.tile([1, BC], F32, tag="tot")
            nc.vector.tensor_add(tot_s[:], psc_s[:], kps_s[:])
            emx = smpool.tile([1, BC], F32, tag="emx")
            nc.scalar.activation(emx[:], tot_s[:], AF.Exp, bias=bc_s[:], scale=-1.0)
            emx1 = smpool.tile([1, BC], F32, tag="emx1")
            nc.vector.tensor_scalar_add(emx1[:], emx[:], 1.0)
            outs = smpool.tile([1, BC], F32, tag="outs")
            nc.vector.reciprocal(outs[:], emx1[:])
            nc.sync.dma_start(out_d.ap().rearrange("b one -> one b"), outs[:])

    nc.compile()

    # Spread SWDGE gathers across the 4 queues. The ucode locks each DMASW
    # semaphore lane to one queue, and Tile assigns lanes round-robin in
    # scheduled order, so derive queue from the assigned lane post-compile.
    import re as _re
    for blk in nc.m.functions[0].blocks:
        for inst in blk.instructions:
            if type(inst).__name__ == "InstDMAGatherAnt":
                for u in inst.sync_info.on_update:
                    m = _re.match(r"DMASW(\d+)_", u.ant_name or "")
                    if m:
                        inst.queue_num = int(m.group(1)) % 4
                        break

    _PROGRAM_CACHE[fast] = nc
    return nc


def _wrap16(flat_idx):
    """int16 index list -> (128, n/16) tile layout: unwrapped[i] =
    tile[i % 16, i // 16], replicated into all 8 16-partition stripes."""
    n = flat_idx.shape[0]
    t = np.zeros((16, n // 16), np.int16)
    t[np.arange(n) % 16, np.arange(n) // 16] = flat_idx
    return np.tile(t, (8, 1))


def _prep_core_inputs(inputs, core, fast):
    """Host-side shard + weight re-layouts for one core."""
    W1 = np.asarray(inputs["W1"], np.float32)
    W2 = np.asarray(inputs["W2"], np.float32)
    Wv = np.asarray(inputs["Wv"], np.float32)
    Wc = np.asarray(inputs["Wc"], np.float32)
    b1 = np.asarray(inputs["b1"], np.float32)
    b2 = np.asarray(inputs["b2"], np.float32)
    bv = np.asarray(inputs["bv"], np.float32)
    bc = np.asarray(inputs["bc"], np.float32)

    sl = slice(core * BC, (core + 1) * BC)
    ev = np.asarray(inputs["batch_event"][sl], np.int64)          # (BC, C)
    feats = np.asarray(inputs["batch_features"][sl], np.float32)  # (BC, NF)
    dists = np.asarray(inputs["batch_distances"][sl], np.float32) # (BC, 9)
    ctx = np.asarray(inputs["batch_context"][sl], np.int64)       # (BC, N, C)

    bf = ml_dtypes.bfloat16
    f8 = ml_dtypes.float8_e4m3

    wc_full = np.zeros((128,), np.float32)
    wc_full[32 : 32 + 9] = Wc[0, 0:9]          # dist_emb block
    wc_full[64 : 64 + NF] = Wc[0, 9 : 9 + NF]  # batch_features block
    wckp = (Wc[0, NF + 9 :] * 0.01).astype(np.float32)  # kp block, 0.01 folded

    m = {}
    if not fast:
        m.update({
            "bvd": bv.reshape(9, 1),
            "wct": wc_full.reshape(-1, 1),
            "wckp": wckp.reshape(1, NK),
            "bcd": -bc.reshape(1, 1),
            "ndsq": np.ascontiguousarray(-(dists * dists).T),
            "featT": np.ascontiguousarray(feats.T),
        })

    if fast:
        # fp8 DoubleRow weight layouts; operands scaled x16, K padded
        # per-component 300 -> 512 as (f, p, beta) = 256f + 2p + beta,
        # H1 units permuted as (P, mm, beta2) = 256P + 2mm + beta2.
        W1pad = np.zeros((C, EP, H1), np.float32)
        for c in range(C):
            W1pad[c, :E, :] = W1[:, E * c : E * (c + 1)].T
        a = W1pad.reshape(C, 2, 128, 2, 2, 128, 2)  # [c,f,p,beta,P,mm,beta2]
        w1t = np.ascontiguousarray(
            a.transpose(2, 0, 1, 3, 4, 6, 5).reshape(128, -1) * FS8
        ).astype(f8)

        a2 = W2.T.reshape(2, 128, 2, 2, 128)  # [P,p,beta,m,mm]
        w2t = np.ascontiguousarray(
            a2.transpose(1, 0, 2, 3, 4).reshape(128, -1) * FS8
        ).astype(f8)

        Wvpad = np.zeros((EP, 9), np.float32)
        Wvpad[:E] = Wv.T
        av = Wvpad.reshape(2, 128, 2, 9)  # [f,p,beta,9]
        wvt = np.ascontiguousarray(
            av.transpose(1, 0, 2, 3).reshape(128, -1) * FS8
        ).astype(f8)

        bb1 = b1.reshape(2, 128, 2)  # [P,p,beta2]
        m["w1t"], m["w2t"], m["wvt"] = w1t, w2t, wvt
        blob = np.zeros((128, 148), np.float32)
        blob[:, 0:4] = np.ascontiguousarray(
            bb1.transpose(1, 0, 2).reshape(128, 4) * FS8
        )
        blob[:, 4:6] = b2.reshape(2, 128).T
        blob[0:9, 6] = bv
        blob[:, 7] = wc_full
        blob[0, 8:19] = wckp
        blob[0, 19] = -bc[0]
        blob[0:9, 20:84] = -(dists * dists).T
        blob[0:NF, 84:148] = feats.T
        m["cblob"] = blob
    else:
        w1t = np.zeros((4 * 384, H1), np.float32)
        for c in range(C):
            w1t[384 * c : 384 * c + E, :] = W1[:, E * c : E * (c + 1)].T
        wvt = np.zeros((4 * 384, 9), np.float32)
        wvt[384 * 1 : 384 * 1 + E, :] = Wv.T  # predicates = component 1
        m["w1t"] = w1t.astype(bf)
        m["w2t"] = np.ascontiguousarray(W2.T).astype(bf)
        m["wvt"] = wvt.astype(bf)
        m["b1d"] = np.ascontiguousarray(b1.reshape(4, 128).T)
        m["b2d"] = np.ascontiguousarray(b2.reshape(2, 128).T)

    if fast:
        table = np.asarray(inputs["event_table"])
        allidx = np.concatenate([ctx.reshape(-1), ev.reshape(-1)])
        uniq, inv = np.unique(allidx, return_inverse=True)
        assert len(uniq) <= CT
        ctab = np.zeros((CT, EP), f8)
        ctab[: len(uniq), :E] = (
            np.asarray(table[uniq], np.float32) * FS8
        ).astype(f8)
        rctx = inv[: ctx.size].astype(np.int16).reshape(BC, N, C)
        rev = inv[ctx.size :].astype(np.int16).reshape(BC, C)

        # context: gather (g, c) of 512 idx with i = s*128 + n
        rc = rctx.reshape(GROUPS, SUBT, N, C)
        cidx = np.concatenate(
            [
                _wrap16(np.ascontiguousarray(rc[g, :, :, c]).reshape(-1))
                for g in range(GROUPS)
                for c in range(C)
            ],
            axis=1,
        )
        # event rows packed K-major on host: col = f*1024 + 256c + 2b + beta
        Epad = np.zeros((128, C, EP), np.float32)
        Epad[:BC, :, :E] = np.asarray(table[ev], np.float32) * FS8
        a_ev = Epad.reshape(128, C, 2, 128, 2)  # [b, c, f, p, beta]
        m["xev"] = np.ascontiguousarray(
            a_ev.transpose(3, 2, 1, 0, 4).reshape(128, 2048)
        ).astype(f8)
        m["ctab"] = ctab
        m["cidx"] = np.ascontiguousarray(cidx)
    else:
        m["table"] = np.ascontiguousarray(
            np.asarray(inputs["event_table"], np.float32)
        )
        m["ctxidx"] = np.ascontiguousarray(
            ctx.astype(np.int32).transpose(1, 0, 2).reshape(128, BC * C)
        )
        m["evidx"] = ev.astype(np.int32)
    return m


def kernel(**inputs) -> np.ndarray:
    # fast path requires every shard's unique row count to fit int16
    fast = True
    ctx = np.asarray(inputs["batch_context"], np.int64)
    ev = np.asarray(inputs["batch_event"], np.int64)
    for core in range(NCORES):
        sl = slice(core * BC, (core + 1) * BC)
        nuniq = len(np.unique(np.concatenate(
            [ctx[sl].reshape(-1), ev[sl].reshape(-1)])))
        if nuniq > CT:
            fast = False
            break
    nc = _build_program(fast)
    in_maps = [_prep_core_inputs(inputs, core, fast) for core in range(NCORES)]
    res = run_bass_kernel_spmd(nc, in_maps, core_ids=list(range(NCORES)))
    return np.concatenate([r["out"] for r in res.results], axis=0)


if __name__ == "__main__":
    nc = _build_program(True)
    print("program built ok")


# revision 47
# speedup vs baseline: 1.0156x; 1.0156x over previous
"""Trainium2 Bass kernel for nn_EventPairCompositionModel.

Strategy (data-parallel over batch, 8 cores, B=512 -> 64 per core):
  - Host compacts the 60MB f32 table per core to the ~24K unique rows its
    shard touches, stored fp8e4m3 (x16 scale), rows padded to 512 elems
    (512B), indices remapped to int16.  SWDGE dma_gather (transpose mode)
    lands rows K-major as 16-bit token pairs: partition p holds elems
    (256f+2p, 256f+2p+1) at free bytes (2i, 2i+1).
  - MLP1/MLP2 run in fp8 with MatmulPerfMode.DoubleRow (256-K per pass,
    2x PE throughput).  The required K/M permutations are absorbed into
    host-side weight layouts; scales (x16 per operand) are folded into
    the activation scale (1/256) and biases.
  - s1 activations write fp8 interleaved pairs (stride-2 bytes) so MLP2
    consumes them directly in DoubleRow layout.
  - Cosine numerators/denominators via small per-b bf16 matmuls that land
    n-on-partitions; norms folded together through one exp(-0.5 ln x).
  - KNRM kernel pooling via ones-matmul partition reductions, distance
    kernel path, final linear + sigmoid, all on-chip.
  - If a shard ever touches >32767 unique rows (can't happen for random
    inputs), falls back to a slow indirect-DMA gather of the full table.
All 8 cores run the identical program on their own batch shard (SPMD, no
collectives); host concatenates the 8 (64,1) outputs.
"""

import numpy as np
import ml_dtypes

import concourse.bacc as bacc
import concourse.bass as bass
import concourse.tile as tile
import concourse.mybir as mybir
from concourse.bass import IndirectOffsetOnAxis
from concourse.bass_utils import run_bass_kernel_spmd
from concourse import library_config

F32 = mybir.dt.float32
BF16 = mybir.dt.bfloat16
F8 = mybir.dt.float8e4
I16 = mybir.dt.int16
I32 = mybir.dt.int32
AF = mybir.ActivationFunctionType
DR = mybir.MatmulPerfMode.DoubleRow

# Problem shapes (hardcoded per spec)
B, N, C, E = 512, 128, 4, 300
V = 50000
H1, H2 = 512, 256
NF, NK = 8, 11
NCORES = 8
BC = B // NCORES          # 64 batches per core
EP = 512                  # padded fp8 row length (512B, two 256-elem chunks)
CE = C * EP               # padded K (2048); real K is 4*300=1200
CT = 32768                # compact table rows (int16-indexable)
GROUPS = (BC * N) // 512  # 16 groups of 512 (b,n) pairs
SUBT = 4                  # 128-pair subtiles per group (s <-> b offset)
EB = 128                  # event-path width (64 real b + 64 junk)
FS8 = 16.0                # fp8 operand scale

MUS = [1.0, 0.9, 0.7, 0.5, 0.3, 0.1, -0.1, -0.3, -0.5, -0.7, -0.9]
SIGMAS = [1e-3] + [0.1] * 10

_PROGRAM_CACHE = {}


def _build_program(fast: bool):
    if fast in _PROGRAM_CACHE:
        return _PROGRAM_CACHE[fast]

    nc = bacc.Bacc("TRN2", target_bir_lowering=False, debug=False, num_swdge_queues=4)

    # ---- DRAM I/O ----
    if fast:
        ctab = nc.dram_tensor("ctab", (CT, EP), F8, kind="ExternalInput")
        cidx = nc.dram_tensor("cidx", (128, GROUPS * C * 32), I16, kind="ExternalInput")
        xev = nc.dram_tensor("xev", (128, 2048), F8, kind="ExternalInput")
        cblob = nc.dram_tensor("cblob", (128, 148), F32, kind="ExternalInput")
        w1t = nc.dram_tensor("w1t", (128, 16 * H1), F8, kind="ExternalInput")
        w2t = nc.dram_tensor("w2t", (128, 4 * H2), F8, kind="ExternalInput")
        wvt = nc.dram_tensor("wvt", (128, 4 * 9), F8, kind="ExternalInput")
    else:
        ctab = nc.dram_tensor("table", (V + 1, E), F32, kind="ExternalInput")
        cidx = nc.dram_tensor("ctxidx", (128, BC * C), I32, kind="ExternalInput")
        eidx = nc.dram_tensor("evidx", (BC, C), I32, kind="ExternalInput")
        w1t = nc.dram_tensor("w1t", (4 * 384, H1), BF16, kind="ExternalInput")
        w2t = nc.dram_tensor("w2t", (H1, H2), BF16, kind="ExternalInput")
        wvt = nc.dram_tensor("wvt", (4 * 384, 9), BF16, kind="ExternalInput")
        b1d = nc.dram_tensor("b1d", (128, 4), F32, kind="ExternalInput")
        b2d = nc.dram_tensor("b2d", (128, 2), F32, kind="ExternalInput")
        bvd = nc.dram_tensor("bvd", (9, 1), F32, kind="ExternalInput")
        wct = nc.dram_tensor("wct", (128, 1), F32, kind="ExternalInput")
        wckp = nc.dram_tensor("wckp", (1, NK), F32, kind="ExternalInput")
        bcd = nc.dram_tensor("bcd", (1, 1), F32, kind="ExternalInput")
        ndsq = nc.dram_tensor("ndsq", (9, BC), F32, kind="ExternalInput")
        featT = nc.dram_tensor("featT", (NF, BC), F32, kind="ExternalInput")
    out_d = nc.dram_tensor("out", (BC, 1), F32, kind="ExternalOutput")

    with tile.TileContext(nc) as tc:
        with (
            tc.tile_pool(name="consts", bufs=1) as cpool,
            tc.tile_pool(name="xg", bufs=4) as xgpool,
            tc.tile_pool(name="xt", bufs=3) as xtpool,
            tc.tile_pool(name="s1", bufs=4) as s1pool,
            tc.tile_pool(name="s2", bufs=4) as s2pool,
            tc.tile_pool(name="csq", bufs=4) as csqpool,
            tc.tile_pool(name="small", bufs=2) as smpool,
            tc.tile_pool(name="pm1", bufs=3, space="PSUM") as pm1,
            tc.tile_pool(name="pm2", bufs=2, space="PSUM") as pm2,
            tc.tile_pool(name="ptn", bufs=1, space="PSUM") as ptn,
            tc.tile_pool(name="pmisc", bufs=2, space="PSUM") as pmisc,
        ):
            # ---- load constants ----
            nc.gpsimd.load_library(library_config.mlp)
            if fast:
                cidx_s = cpool.tile([128, GROUPS * C * 32], I16)
                nc.sync.dma_start(cidx_s[:], cidx.ap())
                w1t_s = cpool.tile([128, 16 * H1], F8)
                nc.scalar.dma_start(w1t_s[:], w1t.ap())
                xeT = cpool.tile([128, 2048], F8)
                nc.sync.dma_start(xeT[:], xev.ap())
                blob_s = cpool.tile([128, 148], F32)
                nc.sync.dma_start(blob_s[:], cblob.ap())
                w2t_s = cpool.tile([128, 4 * H2], F8)
                nc.scalar.dma_start(w2t_s[:], w2t.ap())
                wvt_s = cpool.tile([128, 4 * 9], F8)
                nc.scalar.dma_start(wvt_s[:], wvt.ap())
            else:
                cidx_s = cpool.tile([128, BC * C], I32)
                nc.sync.dma_start(cidx_s[:], cidx.ap())
                eidx_s = cpool.tile([BC, C], I32)
                nc.sync.dma_start(eidx_s[:], eidx.ap())
                w1t_s = cpool.tile([128, 12 * H1], BF16)
                nc.sync.dma_start(
                    w1t_s[:].rearrange("p (t m) -> p t m", t=12),
                    w1t.ap().rearrange("(t p) m -> p t m", p=128),
                )
                w2t_s = cpool.tile([128, 4 * H2], BF16)
                nc.scalar.dma_start(
                    w2t_s[:].rearrange("p (t m) -> p t m", t=4),
                    w2t.ap().rearrange("(t p) m -> p t m", p=128),
                )
                wvt_s = cpool.tile([128, 12 * 9], BF16)
                nc.scalar.dma_start(
                    wvt_s[:].rearrange("p (t m) -> p t m", t=12),
                    wvt.ap().rearrange("(t p) m -> p t m", p=128),
                )
            b1_s = cpool.tile([128, 4], F32)
            b2_s = cpool.tile([128, 2], F32)
            bv_s = cpool.tile([9, 1], F32)
            wct_s = cpool.tile([128, 1], F32)
            wckp_s = cpool.tile([1, NK], F32)
            bc_s = cpool.tile([1, 1], F32)
            ndsq_s = cpool.tile([9, BC], F32)
            feat_s = cpool.tile([128, BC], F32)
            nc.vector.memset(feat_s[:], 0.0)
            if fast:
                nc.vector.tensor_scalar_add(b1_s[:], blob_s[:, 0:4], 0.0)
                nc.vector.tensor_scalar_add(b2_s[:], blob_s[:, 4:6], 0.0)
                nc.vector.tensor_scalar_add(bv_s[:], blob_s[0:9, 6:7], 0.0)
                nc.vector.tensor_scalar_add(wct_s[:], blob_s[:, 7:8], 0.0)
                nc.vector.tensor_scalar_add(wckp_s[:], blob_s[0:1, 8:19], 0.0)
                nc.vector.tensor_scalar_add(bc_s[:], blob_s[0:1, 19:20], 0.0)
                nc.vector.tensor_scalar_add(ndsq_s[:], blob_s[0:9, 20:84], 0.0)
                nc.vector.tensor_scalar_add(
                    feat_s[64 : 64 + NF, :], blob_s[0:NF, 84:148], 0.0
                )
            else:
                nc.sync.dma_start(b1_s[:], b1d.ap())
                nc.sync.dma_start(b2_s[:], b2d.ap())
                nc.sync.dma_start(bv_s[:], bvd.ap())
                nc.sync.dma_start(wct_s[:], wct.ap())
                nc.sync.dma_start(wckp_s[:], wckp.ap())
                nc.sync.dma_start(bc_s[:], bcd.ap())
                nc.sync.dma_start(ndsq_s[:], ndsq.ap())
                nc.sync.dma_start(feat_s[64 : 64 + NF, :], featT.ap())
            ones_s = cpool.tile([128, 1], BF16)
            nc.vector.memset(ones_s[:], 1.0)
            onesrow_s = cpool.tile([1, 128], F32)
            nc.vector.memset(onesrow_s[:], 1.0)
            onesf_s = cpool.tile([128, 1], F32)
            nc.vector.memset(onesf_s[:], 1.0)
            eps_s = cpool.tile([128, 1], F32)
            nc.vector.memset(eps_s[:], 1e-20)
            mub_s = cpool.tile([128, NK], F32)
            coef_s = cpool.tile([128, NK], F32)
            for k in range(NK):
                nc.vector.memset(mub_s[:, k : k + 1], -MUS[k])
                nc.vector.memset(
                    coef_s[:, k : k + 1], -1.0 / (2.0 * SIGMAS[k] ** 2)
                )

            eh2 = [
                cpool.tile([128, EB], BF16, tag=f"eh2_{k}", name=f"eh2_{k}")
                for k in range(2)
            ]

            if fast:
                # ---- fp8 DoubleRow access-pattern helpers ----
                def w1_lhs(c, f, m):
                    # [p, beta(2), mm(128)] for K-chunk (c,f), M-tile m
                    v = w1t_s[:].rearrange(
                        "p (cf beta u) -> p cf beta u", beta=2, u=H1
                    )
                    return v[:, c * 2 + f, :, 128 * m : 128 * (m + 1)]

                def w2_lhs(P, m):
                    v = w2t_s[:].rearrange(
                        "p (q beta u) -> p q beta u", beta=2, u=H2
                    )
                    return v[:, P, :, 128 * m : 128 * (m + 1)]

                def wv_lhs(f):
                    v = wvt_s[:].rearrange(
                        "p (f beta u) -> p f beta u", f=2, u=9
                    )
                    return v[:, f, :, :]

                def issue_group_gather(g):
                    xt = xtpool.tile([128, C * 2048], F8, tag="xt", name="xt")
                    for c in range(C):
                        nc.gpsimd.dma_gather(
                            out_ap=xt[:].rearrange(
                                "p (c j i) -> p c j i", c=C, j=4
                            )[:, c, :, :],
                            in_ap=ctab.ap(),
                            idxs_ap=cidx_s[
                                :, 32 * (C * g + c) : 32 * (C * g + c + 1)
                            ],
                            num_idxs=512,
                            num_idxs_reg=512,
                            elem_size=EP,
                            transpose=True,
                        )
                    return xt


                def xe_rhs(c, f):
                    # [p, beta(2), n(128)]: comp c chunk f over event cols
                    v = xeT[:].rearrange(
                        "p (f i beta) -> p f beta i", f=2, beta=2
                    )
                    return v[:, f, :, 128 * c : 128 * (c + 1)]

                s1e = cpool.tile([128, 512], F8)
                for m in range(4):
                    pe = pmisc.tile([128, EB], F32, tag="pmisc", name="pe")
                    kk = 0
                    for c in range(C):
                        for f in range(2):
                            nc.tensor.matmul(
                                pe[:], w1_lhs(c, f, m), xe_rhs(c, f),
                                start=(kk == 0), stop=(kk == 7), perf_mode=DR,
                            )
                            kk += 1
                    P, bb = divmod(m, 2)
                    o = s1e[:].rearrange(
                        "p (q i beta) -> p q beta i", q=2, beta=2
                    )[:, P, bb, :]
                    nc.scalar.activation(
                        o, pe[:], AF.Relu, bias=b1_s[:, m : m + 1],
                        scale=1.0 / FS8,
                    )

                def s1e_rhs(P):
                    v = s1e[:].rearrange(
                        "p (q i beta) -> p q beta i", q=2, beta=2
                    )
                    return v[:, P, :, :]

                for m in range(2):
                    pe2 = pmisc.tile([128, EB], F32, tag="pmisc", name="pe2")
                    for P in range(2):
                        nc.tensor.matmul(
                            pe2[:], w2_lhs(P, m), s1e_rhs(P),
                            start=(P == 0), stop=(P == 1), perf_mode=DR,
                        )
                    nc.scalar.activation(
                        eh2[m][:], pe2[:], AF.Relu, bias=b2_s[:, m : m + 1],
                        scale=1.0 / (FS8 * FS8),
                    )

                # variances -> dist_emb rows 32..40 of feat_s
                # (plain fp8 matmuls; DoubleRow rejects the 9-wide ldweights)
                pv = pmisc.tile([9, EB], F32, tag="pmisc", name="pv")
                for f in range(2):
                    for bb in range(2):
                        nc.tensor.matmul(
                            pv[:],
                            wv_lhs(f)[:, bb, :],
                            xe_rhs(1, f)[:, bb, :],
                            start=(f == 0 and bb == 0),
                            stop=(f == 1 and bb == 1),
                        )
                ez_s = smpool.tile([9, EB], F32)
                nc.scalar.activation(
                    ez_s[:], pv[:], AF.Exp, bias=bv_s[:],
                    scale=1.0 / (FS8 * FS8),
                )
            else:
                # ---- slow-path event MLP (bf16, padded-384 K layout) ----
                xe = cpool.tile([EB, 4 * 384], BF16)
                nc.vector.memset(xe[:], 0.0)
                nc.gpsimd.indirect_dma_start(
                    out=xe[0:BC, :].rearrange("p (c e) -> p c e", c=C)[:, :, 0:E],
                    out_offset=None,
                    in_=ctab.ap(),
                    in_offset=IndirectOffsetOnAxis(ap=eidx_s[:], axis=0),
                )
                xeT = cpool.tile([128, 12 * EB], BF16)
                nc.sync.dma_start_transpose(
                    xeT[:].rearrange("p (j i) -> p j i", j=12), xe[:]
                )

                def xeT_k(j):
                    return xeT[:, EB * j : EB * (j + 1)]

                s1e = cpool.tile([128, 4 * EB], BF16)
                for m in range(4):
                    pe = pmisc.tile([128, EB], F32, tag="pmisc", name="pe")
                    for j in range(12):
                        nc.tensor.matmul(
                            pe[:],
                            w1t_s[:, H1 * j + 128 * m : H1 * j + 128 * m + 128],
                            xeT_k(j),
                            start=(j == 0),
                            stop=(j == 11),
                        )
                    nc.scalar.activation(
                        s1e[:, EB * m : EB * (m + 1)], pe[:], AF.Relu,
                        bias=b1_s[:, m : m + 1],
                    )

                for m in range(2):
                    pe2 = pmisc.tile([128, EB], F32, tag="pmisc", name="pe2")
                    for j in range(4):
                        nc.tensor.matmul(
                            pe2[:],
                            w2t_s[:, H2 * j + 128 * m : H2 * j + 128 * m + 128],
                            s1e[:, EB * j : EB * (j + 1)],
                            start=(j == 0),
                            stop=(j == 3),
                        )
                    nc.scalar.activation(
                        eh2[m][:], pe2[:], AF.Relu, bias=b2_s[:, m : m + 1]
                    )

                pv = pmisc.tile([9, EB], F32, tag="pmisc", name="pv")
                for j in range(12):
                    nc.tensor.matmul(
                        pv[:],
                        wvt_s[:, 9 * j : 9 * (j + 1)],
                        xeT_k(j),
                        start=(j == 0),
                        stop=(j == 11),
                    )
                ez_s = smpool.tile([9, EB], F32)
                nc.scalar.activation(ez_s[:], pv[:], AF.Exp, bias=bv_s[:])

            # softplus -> variances -> dist kernel features (shared)
            ez1_s = smpool.tile([9, EB], F32)
            nc.vector.tensor_scalar_add(ez1_s[:], ez_s[:], 1.0)
            var_s = smpool.tile([9, EB], F32)
            nc.scalar.activation(var_s[:], ez1_s[:], AF.Ln)
            rv_s = smpool.tile([9, EB], F32)
            nc.vector.reciprocal(rv_s[:], var_s[:])
            q_s = smpool.tile([9, BC], F32)
            nc.vector.tensor_mul(q_s[:], ndsq_s[:], rv_s[:, 0:BC])
            nc.scalar.activation(feat_s[32:41, :], q_s[:], AF.Exp)

            # |e|^2 per b, broadcast to all 128 partitions via outer product
            esq = [
                smpool.tile([128, EB], BF16, tag=f"esq_{k}", name=f"esq_{k}")
                for k in range(2)
            ]
            for k in range(2):
                nc.vector.tensor_mul(esq[k][:], eh2[k][:], eh2[k][:])
            pne = pmisc.tile([1, EB], F32, tag="pmisc", name="pne")
            for k in range(2):
                nc.tensor.matmul(
                    pne[:], ones_s[:], esq[k][:], start=(k == 0), stop=(k == 1)
                )
            ne2_s = smpool.tile([1, BC], F32)
            nc.scalar.copy(ne2_s[:], pne[:, 0:BC])
            pne2bc = pmisc.tile([128, BC], F32, tag="pmisc", name="pne2bc")
            nc.tensor.matmul(
                pne2bc[:], onesrow_s[:], ne2_s[:], start=True, stop=True
            )
            ne2bc_s = cpool.tile([128, BC], F32)
            nc.scalar.copy(ne2bc_s[:], pne2bc[:])

            # feature-vector part of the score (ready before the groups)
            psc = pmisc.tile([1, BC], F32, tag="pmisc", name="psc")
            nc.tensor.matmul(psc[:], wct_s[:], feat_s[:], start=True, stop=True)
            psc_s = cpool.tile([1, BC], F32)
            nc.vector.tensor_scalar_add(psc_s[:], psc[:], 0.0)

            # persistent SBUF accumulators, n on partitions, b on free
            traw_s = cpool.tile([128, BC], F32)
            ncsq_s = cpool.tile([128, BC], F32)
            kps_s = cpool.tile([1, BC], F32)

            def do_pool(b0, b1):
                """RBF kernel pooling for b in [b0, b1)."""
                QB = b1 - b0
                sl = slice(b0, b1)
                prodn = smpool.tile([128, QB], F32, tag="prodn", name="prodn")
                nc.vector.tensor_mul(prodn[:], ncsq_s[:, sl], ne2bc_s[:, sl])
                lnp = smpool.tile([128, QB], F32, tag="lnp", name="lnp")
                nc.scalar.activation(lnp[:], prodn[:], AF.Ln, bias=eps_s[:])
                nrmf = smpool.tile([128, QB], F32, tag="nrmf", name="nrmf")
                nc.scalar.activation(nrmf[:], lnp[:], AF.Exp, scale=-0.5)
                transq = smpool.tile([128, QB], F32, tag="transq", name="transq")
                nc.vector.tensor_mul(transq[:], traw_s[:, sl], nrmf[:])
                dkb = smpool.tile([128, NK * QB], F32, tag="dkb", name="dkb")
                nc.vector.tensor_tensor(
                    out=dkb[:].rearrange("p (k b) -> p k b", k=NK),
                    in0=transq[:][:, None, :].broadcast_to([128, NK, QB]),
                    in1=mub_s[:][:, :, None].broadcast_to([128, NK, QB]),
                    op=mybir.AluOpType.add,
                )
                sqkb = smpool.tile([128, NK * QB], F32, tag="sqkb", name="sqkb")
                nc.vector.tensor_mul(sqkb[:], dkb[:], dkb[:])
                argkb = smpool.tile([128, NK * QB], F32, tag="argkb", name="argkb")
                nc.vector.tensor_tensor(
                    out=argkb[:].rearrange("p (k b) -> p k b", k=NK),
                    in0=sqkb[:].rearrange("p (k b) -> p k b", k=NK),
                    in1=coef_s[:][:, :, None].broadcast_to([128, NK, QB]),
                    op=mybir.AluOpType.mult,
                )
                argc = smpool.tile([128, NK * QB], F32, tag="argc", name="argc")
                nc.vector.tensor_scalar_max(argc[:], argkb[:], -87.0)
                ekb = smpool.tile([128, NK * QB], BF16, tag="ekb", name="ekb")
                nc.scalar.activation(ekb[:], argc[:], AF.Exp)
                kpc = smpool.tile([1, NK * QB], F32, tag="kpc", name="kpc")
                for lo in range(0, NK * QB, 512):
                    hi = min(lo + 512, NK * QB)
                    pp = pmisc.tile([1, hi - lo], F32, tag="pmisc", name="pp")
                    nc.tensor.matmul(
                        pp[:], ones_s[:], ekb[:, lo:hi], start=True, stop=True
                    )
                    nc.vector.tensor_scalar_max(kpc[:, lo:hi], pp[:], 1e-10)
                kpl = smpool.tile([1, NK * QB], F32, tag="kpl", name="kpl")
                nc.scalar.activation(kpl[:], kpc[:], AF.Ln)
                kpw = smpool.tile([1, QB * NK], F32, tag="kpw", name="kpw")
                nc.vector.tensor_tensor(
                    out=kpw[:].rearrange("p (b k) -> p b k", b=QB),
                    in0=kpl[:].rearrange("p (k b) -> p b k", k=NK),
                    in1=wckp_s[:][:, None, :].broadcast_to([1, QB, NK]),
                    op=mybir.AluOpType.mult,
                )
                nc.vector.reduce_sum(
                    out=kps_s[:, sl],
                    in_=kpw[:].rearrange("p (b k) -> p b k", b=QB),
                    axis=mybir.AxisListType.X,
                )

            # ---- context groups ----
            for g in range(GROUPS):
                if fast:
                    xt = issue_group_gather(g)

                    def xt_rhs(c, f):
                        v = xt[:].rearrange(
                            "p (c f i beta) -> p c f beta i", c=C, f=2, beta=2
                        )
                        return v[:, c, f, :, :]

                    s1 = s1pool.tile([128, 2048], F8, tag="s1", name="s1")
                    for m in range(4):
                        p1 = pm1.tile([128, 512], F32)
                        kk = 0
                        for c in range(C):
                            for f in range(2):
                                nc.tensor.matmul(
                                    p1[:], w1_lhs(c, f, m), xt_rhs(c, f),
                                    start=(kk == 0), stop=(kk == 7),
                                    perf_mode=DR,
                                )
                                kk += 1
                        P, bb = divmod(m, 2)
                        o = s1[:].rearrange(
                            "p (q i beta) -> p q beta i", q=2, beta=2
                        )[:, P, bb, :]
                        nc.scalar.activation(
                            o, p1[:], AF.Relu, bias=b1_s[:, m : m + 1],
                            scale=1.0 / FS8,
                        )

                    def s1_rhs(P):
                        v = s1[:].rearrange(
                            "p (q i beta) -> p q beta i", q=2, beta=2
                        )
                        return v[:, P, :, :]

                    s2 = [
                        s2pool.tile(
                            [128, 512], BF16, tag=f"s2_{m}", name=f"s2_{m}"
                        )
                        for m in range(2)
                    ]
                    for m in range(2):
                        p2 = pm2.tile([128, 512], F32)
                        for P in range(2):
                            nc.tensor.matmul(
                                p2[:], w2_lhs(P, m), s1_rhs(P),
                                start=(P == 0), stop=(P == 1), perf_mode=DR,
                            )
                        nc.scalar.activation(
                            s2[m][:], p2[:], AF.Relu, bias=b2_s[:, m : m + 1],
                            scale=1.0 / (FS8 * FS8),
                        )

                    csq = [
                        csqpool.tile(
                            [128, 512], BF16, tag=f"csq_{m}", name=f"csq_{m}"
                        )
                        for m in range(2)
                    ]
                    for m in range(2):
                        nc.vector.tensor_mul(csq[m][:], s2[m][:], s2[m][:])

                    # raw dots and |c|^2, n on partitions, one column per b
                    pTN = ptn.tile([128, 2 * SUBT], F32, tag="pTN", name="pTN")
                    pT = pTN[:, 0:SUBT]
                    pN = pTN[:, SUBT : 2 * SUBT]
                    for s in range(SUBT):
                        b = SUBT * g + s
                        for k in range(2):
                            nc.tensor.matmul(
                                pT[:, s : s + 1],
                                s2[k][:, 128 * s : 128 * (s + 1)],
                                eh2[k][:, b : b + 1],
                                start=(k == 0),
                                stop=(k == 1),
                            )
                        for k in range(2):
                            nc.tensor.matmul(
                                pN[:, s : s + 1],
                                csq[k][:, 128 * s : 128 * (s + 1)],
                                ones_s[:],
                                start=(k == 0),
                                stop=(k == 1),
                            )
                else:
                    xg = xgpool.tile([128, SUBT * 4 * 384], BF16)
                    nc.vector.memset(
                        xg[:].rearrange("p (q e) -> p q e", e=384)[:, :, E:384],
                        0.0,
                    )
                    for s in range(SUBT):
                        nc.gpsimd.indirect_dma_start(
                            out=xg[:]
                            .rearrange("p (q c e) -> p q c e", q=SUBT, c=C)[
                                :, s, :, 0:E
                            ],
                            out_offset=None,
                            in_=ctab.ap(),
                            in_offset=IndirectOffsetOnAxis(
                                ap=cidx_s[
                                    :, (SUBT * g + s) * C : (SUBT * g + s + 1) * C
                                ],
                                axis=0,
                            ),
                        )
                    xt = xtpool.tile([128, 12 * 512], BF16)
                    for s in range(SUBT):
                        nc.sync.dma_start_transpose(
                            xt[:].rearrange(
                                "p (j z i) -> p j z i", j=12, z=SUBT
                            )[:, :, s, :],
                            xg[:, 1536 * s : 1536 * (s + 1)],
                        )

                    def xt_k(j):
                        return xt[:, 512 * j : 512 * (j + 1)]

                    s1t = [
                        s1pool.tile(
                            [128, 512], BF16, tag=f"s1_{m}", name=f"s1_{m}"
                        )
                        for m in range(4)
                    ]
                    for m in range(4):
                        p1 = pm1.tile([128, 512], F32)
                        for j in range(12):
                            nc.tensor.matmul(
                                p1[:],
                                w1t_s[
                                    :, H1 * j + 128 * m : H1 * j + 128 * m + 128
                                ],
                                xt_k(j),
                                start=(j == 0),
                                stop=(j == 11),
                            )
                        nc.scalar.activation(
                            s1t[m][:], p1[:], AF.Relu, bias=b1_s[:, m : m + 1]
                        )

                    s2 = [
                        s2pool.tile(
                            [128, 512], BF16, tag=f"s2_{m}", name=f"s2_{m}"
                        )
                        for m in range(2)
                    ]
                    for m in range(2):
                        p2 = pm2.tile([128, 512], F32)
                        for j in range(4):
                            nc.tensor.matmul(
                                p2[:],
                                w2t_s[
                                    :, H2 * j + 128 * m : H2 * j + 128 * m + 128
                                ],
                                s1t[j][:],
                                start=(j == 0),
                                stop=(j == 3),
                            )
                        nc.scalar.activation(
                            s2[m][:], p2[:], AF.Relu, bias=b2_s[:, m : m + 1]
                        )

                    csq = [
                        csqpool.tile(
                            [128, 512], BF16, tag=f"csq_{m}", name=f"csq_{m}"
                        )
                        for m in range(2)
                    ]
                    for m in range(2):
                        nc.vector.tensor_mul(csq[m][:], s2[m][:], s2[m][:])

                    # raw dots and |c|^2, n on partitions, one column per b
                    pTN = ptn.tile([128, 2 * SUBT], F32, tag="pTN", name="pTN")
                    pT = pTN[:, 0:SUBT]
                    pN = pTN[:, SUBT : 2 * SUBT]
                    for s in range(SUBT):
                        b = SUBT * g + s
                        for k in range(2):
                            nc.tensor.matmul(
                                pT[:, s : s + 1],
                                s2[k][:, 128 * s : 128 * (s + 1)],
                                eh2[k][:, b : b + 1],
                                start=(k == 0),
                                stop=(k == 1),
                            )
                        for k in range(2):
                            nc.tensor.matmul(
                                pN[:, s : s + 1],
                                csq[k][:, 128 * s : 128 * (s + 1)],
                                ones_s[:],
                                start=(k == 0),
                                stop=(k == 1),
                            )
                nc.vector.tensor_scalar_add(
                    traw_s[:, SUBT * g : SUBT * (g + 1)], pT, 0.0
                )
                nc.vector.tensor_scalar_add(
                    ncsq_s[:, SUBT * g : SUBT * (g + 1)], pN, 0.0
                )
                if fast and g == GROUPS - 2:
                    # prime the Ln/Exp activation tables during the last
                    # group's matmul window so the pooling tail finds them
                    # resident (saves 2x 1.3us table loads on the critical path)
                    dex = smpool.tile([1, 1], F32, tag="dex", name="dex")
                    nc.scalar.activation(dex[:], eps_s[0:1, :], AF.Exp)
                    dln = smpool.tile([1, 1], F32, tag="dln", name="dln")
                    nc.scalar.activation(dln[:], eps_s[0:1, :], AF.Ln)
            do_pool(0, BC // 2)
            do_pool(BC // 2, BC)

            # ---- final score ----
            tot_s = smpool.tile([1, BC], F32, tag="tot")
            nc.vector.tensor_add(tot_s[:], psc_s[:], kps_s[:])
            emx = smpool.tile([1, BC], F32, tag="emx")
            nc.scalar.activation(emx[:], tot_s[:], AF.Exp, bias=bc_s[:], scale=-1.0)
            emx1 = smpool.tile([1, BC], F32, tag="emx1")
            nc.vector.tensor_scalar_add(emx1[:], emx[:], 1.0)
            outs = smpool.tile([1, BC], F32, tag="outs")
            nc.vector.reciprocal(outs[:], emx1[:])
            nc.sync.dma_start(out_d.ap().rearrange("b one -> one b"), outs[:])

    nc.compile()

    # Spread SWDGE gathers across the 4 queues. The ucode locks each DMASW
    # semaphore lane to one queue, and Tile assigns lanes round-robin in
    # scheduled order, so derive queue from the assigned lane post-compile.
    import re as _re
    for blk in nc.m.functions[0].blocks:
        for inst in blk.instructions:
            if type(inst).__name__ == "InstDMAGatherAnt":
                for u in inst.sync_info.on_update:
                    m = _re.match(r"DMASW(\d+)_", u.ant_name or "")
                    if m:
                        inst.queue_num = int(m.group(1)) % 4
                        break

    _PROGRAM_CACHE[fast] = nc
    return nc


def _wrap16(flat_idx):
    """int16 index list -> (128, n/16) tile layout: unwrapped[i] =
    tile[i % 16, i // 16], replicated into all 8 16-partition stripes."""
    n = flat_idx.shape[0]
    t = np.zeros((16, n // 16), np.int16)
    t[np.arange(n) % 16, np.arange(n) // 16] = flat_idx
    return np.tile(t, (8, 1))


def _prep_core_inputs(inputs, core, fast):
    """Host-side shard + weight re-layouts for one core."""
    W1 = np.asarray(inputs["W1"], np.float32)
    W2 = np.asarray(inputs["W2"], np.float32)
    Wv = np.asarray(inputs["Wv"], np.float32)
    Wc = np.asarray(inputs["Wc"], np.float32)
    b1 = np.asarray(inputs["b1"], np.float32)
    b2 = np.asarray(inputs["b2"], np.float32)
    bv = np.asarray(inputs["bv"], np.float32)
    bc = np.asarray(inputs["bc"], np.float32)

    sl = slice(core * BC, (core + 1) * BC)
    ev = np.asarray(inputs["batch_event"][sl], np.int64)          # (BC, C)
    feats = np.asarray(inputs["batch_features"][sl], np.float32)  # (BC, NF)
    dists = np.asarray(inputs["batch_distances"][sl], np.float32) # (BC, 9)
    ctx = np.asarray(inputs["batch_context"][sl], np.int64)       # (BC, N, C)

    bf = ml_dtypes.bfloat16
    f8 = ml_dtypes.float8_e4m3

    wc_full = np.zeros((128,), np.float32)
    wc_full[32 : 32 + 9] = Wc[0, 0:9]          # dist_emb block
    wc_full[64 : 64 + NF] = Wc[0, 9 : 9 + NF]  # batch_features block
    wckp = (Wc[0, NF + 9 :] * 0.01).astype(np.float32)  # kp block, 0.01 folded

    m = {}
    if not fast:
        m.update({
            "bvd": bv.reshape(9, 1),
            "wct": wc_full.reshape(-1, 1),
            "wckp": wckp.reshape(1, NK),
            "bcd": -bc.reshape(1, 1),
            "ndsq": np.ascontiguousarray(-(dists * dists).T),
            "featT": np.ascontiguousarray(feats.T),
        })

    if fast:
        # fp8 DoubleRow weight layouts; operands scaled x16, K padded
        # per-component 300 -> 512 as (f, p, beta) = 256f + 2p + beta,
        # H1 units permuted as (P, mm, beta2) = 256P + 2mm + beta2.
        W1pad = np.zeros((C, EP, H1), np.float32)
        for c in range(C):
            W1pad[c, :E, :] = W1[:, E * c : E * (c + 1)].T
        a = W1pad.reshape(C, 2, 128, 2, 2, 128, 2)  # [c,f,p,beta,P,mm,beta2]
        w1t = np.ascontiguousarray(
            a.transpose(2, 0, 1, 3, 4, 6, 5).reshape(128, -1) * FS8
        ).astype(f8)

        a2 = W2.T.reshape(2, 128, 2, 2, 128)  # [P,p,beta,m,mm]
        w2t = np.ascontiguousarray(
            a2.transpose(1, 0, 2, 3, 4).reshape(128, -1) * FS8
        ).astype(f8)

        Wvpad = np.zeros((EP, 9), np.float32)
        Wvpad[:E] = Wv.T
        av = Wvpad.reshape(2, 128, 2, 9)  # [f,p,beta,9]
        wvt = np.ascontiguousarray(
            av.transpose(1, 0, 2, 3).reshape(128, -1) * FS8
        ).astype(f8)

        bb1 = b1.reshape(2, 128, 2)  # [P,p,beta2]
        m["w1t"], m["w2t"], m["wvt"] = w1t, w2t, wvt
        blob = np.zeros((128, 148), np.float32)
        blob[:, 0:4] = np.ascontiguousarray(
            bb1.transpose(1, 0, 2).reshape(128, 4) * FS8
        )
        blob[:, 4:6] = b2.reshape(2, 128).T
        blob[0:9, 6] = bv
        blob[:, 7] = wc_full
        blob[0, 8:19] = wckp
        blob[0, 19] = -bc[0]
        blob[0:9, 20:84] = -(dists * dists).T
        blob[0:NF, 84:148] = feats.T
        m["cblob"] = blob
    else:
        w1t = np.zeros((4 * 384, H1), np.float32)
        for c in range(C):
            w1t[384 * c : 384 * c + E, :] = W1[:, E * c : E * (c + 1)].T
        wvt = np.zeros((4 * 384, 9), np.float32)
        wvt[384 * 1 : 384 * 1 + E, :] = Wv.T  # predicates = component 1
        m["w1t"] = w1t.astype(bf)
        m["w2t"] = np.ascontiguousarray(W2.T).astype(bf)
        m["wvt"] = wvt.astype(bf)
        m["b1d"] = np.ascontiguousarray(b1.reshape(4, 128).T)
        m["b2d"] = np.ascontiguousarray(b2.reshape(2, 128).T)

    if fast:
        table = np.asarray(inputs["event_table"])
        allidx = np.concatenate([ctx.reshape(-1), ev.reshape(-1)])
        uniq, inv = np.unique(allidx, return_inverse=True)
        assert len(uniq) <= CT
        ctab = np.zeros((CT, EP), f8)
        ctab[: len(uniq), :E] = (
            np.asarray(table[uniq], np.float32) * FS8
        ).astype(f8)
        rctx = inv[: ctx.size].astype(np.int16).reshape(BC, N, C)
        rev = inv[ctx.size :].astype(np.int16).reshape(BC, C)

        # context: gather (g, c) of 512 idx with i = s*128 + n
        rc = rctx.reshape(GROUPS, SUBT, N, C)
        cidx = np.concatenate(
            [
                _wrap16(np.ascontiguousarray(rc[g, :, :, c]).reshape(-1))
                for g in range(GROUPS)
                for c in range(C)
            ],
            axis=1,
        )
        # event rows packed K-major on host: col = f*1024 + 256c + 2b + beta
        Epad = np.zeros((128, C, EP), np.float32)
        Epad[:BC, :, :E] = np.asarray(table[ev], np.float32) * FS8
        a_ev = Epad.reshape(128, C, 2, 128, 2)  # [b, c, f, p, beta]
        m["xev"] = np.ascontiguousarray(
            a_ev.transpose(3, 2, 1, 0, 4).reshape(128, 2048)
        ).astype(f8)
        m["ctab"] = ctab
        m["cidx"] = np.ascontiguousarray(cidx)
    else:
        m["table"] = np.ascontiguousarray(
            np.asarray(inputs["event_table"], np.float32)
        )
        m["ctxidx"] = np.ascontiguousarray(
            ctx.astype(np.int32).transpose(1, 0, 2).reshape(128, BC * C)
        )
        m["evidx"] = ev.astype(np.int32)
    return m


def kernel(**inputs) -> np.ndarray:
    # fast path requires every shard's unique row count to fit int16
    fast = True
    ctx = np.asarray(inputs["batch_context"], np.int64)
    ev = np.asarray(inputs["batch_event"], np.int64)
    for core in range(NCORES):
        sl = slice(core * BC, (core + 1) * BC)
        nuniq = len(np.unique(np.concatenate(
            [ctx[sl].reshape(-1), ev[sl].reshape(-1)])))
        if nuniq > CT:
            fast = False
            break
    nc = _build_program(fast)
    in_maps = [_prep_core_inputs(inputs, core, fast) for core in range(NCORES)]
    res = run_bass_kernel_spmd(nc, in_maps, core_ids=list(range(NCORES)))
    return np.concatenate([r["out"] for r in res.results], axis=0)


if __name__ == "__main__":
    nc = _build_program(True)
    print("program built ok")
